# revision 16
# baseline (speedup 1.0000x reference)
"""MinibatchDiscrimination kernel for Trainium2 (8 NeuronCores, SPMD).

Math: Ms = (x @ W).reshape(B, 128, 16)
      norm[b,i,j] = sum_d |Ms[b,i,d] - Ms[b,j,d]|
      out[b,i]    = sum_j exp(-norm[b,i,j])

Sharding: data-parallel over batch B across 8 cores (256 samples each).
W replicated. Host pre-casts to bf16 and pre-transposes x so the
stationary matmul operand loads without an on-device transpose.

Pairwise stage (per 128-sample tile, partitions = batch):
  - symmetry: for j-block [j0, j0+JB) compute only i in [j0, 128).
    Mirrored contributions for the cross region (i >= j0+JB) are added
    to O[j] via a transposed reduce.
  - DVE does the broadcast subtract at bf16 2x; ScalarE applies |.|
    in place; the d-reduction is a 4-level pairwise add tree on DVE
    (bf16 at 2x, final level fp32); ScalarE applies exp(-norm); DVE
    reduces over j into per-block slices, then once at the end.
"""

import os
import sys

sys.path.insert(0, "/opt/trn_rl_repo")
os.environ.setdefault("MYCRO_LOCAL_CACHE", "1")

import numpy as np
from ml_dtypes import bfloat16

import concourse.bacc as bacc
import concourse.bass as bass
import concourse.tile as tile
from concourse import mybir
from concourse.bass_utils import run_bass_kernel_spmd

B, F, K, D = 2048, 2048, 128, 16
NCORES = 8
BL = B // NCORES          # 256 rows per core
P = 128                   # partitions
NBT = BL // P             # 2 batch tiles per core
FB = F // P               # 16 contraction blocks
ND = K * D                # 2048 output cols of the matmul
JB = 4                    # j-block size in the pairwise stage
NJB = K // JB             # 32 j-blocks

_BF16 = mybir.dt.bfloat16
_F32 = mybir.dt.float32


def _build_nc():
    nc = bacc.Bacc("TRN2", target_bir_lowering=False, debug=False)
    xt = nc.dram_tensor("xt", [F, BL], _BF16, kind="ExternalInput")
    w = nc.dram_tensor("w", [F, ND], _BF16, kind="ExternalInput")
    out = nc.dram_tensor("out", [BL, K], _F32, kind="ExternalOutput")

    with tile.TileContext(nc) as tc:
        with (
            tc.tile_pool(name="const", bufs=1) as const_pool,
            tc.tile_pool(name="work", bufs=2) as work,
            tc.tile_pool(name="acc", bufs=1) as acc,
            tc.tile_pool(name="small", bufs=3) as small,
            tc.tile_pool(name="psum", bufs=2, space="PSUM") as psum_pool,
        ):
            w_sb = const_pool.tile([P, FB, ND], _BF16)
            xt_sb = const_pool.tile([P, FB, BL], _BF16)
            w_r = w.rearrange("(fb p) n -> p fb n", p=P)
            xt_r = xt.rearrange("(fb p) b -> p fb b", p=P)
            for fb in range(FB):
                nc.sync.dma_start(out=w_sb[:, fb, :], in_=w_r[:, fb, :])
                nc.sync.dma_start(out=xt_sb[:, fb, :], in_=xt_r[:, fb, :])

            for t in range(NBT):
                # ---- Ms = x @ W for this 128-sample tile ----
                ms = work.tile([P, K, D], _BF16, tag="ms")
                ms_flat = ms.rearrange("p k d -> p (k d)")
                psums = [
                    psum_pool.tile([P, 512], _F32, tag=f"ps{n}", name=f"ps{n}")
                    for n in range(4)
                ]
                for fb in range(FB):
                    lhsT = xt_sb[:, fb, t * P : (t + 1) * P]
                    for n in range(4):
                        nc.tensor.matmul(
                            psums[n],
                            lhsT,
                            w_sb[:, fb, n * 512 : (n + 1) * 512],
                            start=(fb == 0),
                            stop=(fb == FB - 1),
                        )
                for n in range(4):
                    nc.scalar.copy(
                        out=ms_flat[:, n * 512 : (n + 1) * 512], in_=psums[n][:, :]
                    )

                # ---- pairwise L1 + exp + sum over j (symmetric half) ----
                esum = acc.tile([P, K, NJB], _F32, tag="esum")
                esum2 = acc.tile([P, K], _F32, tag="esum2")
                nc.gpsimd.memset(esum, 0.0)
                nc.gpsimd.memset(esum2, 0.0)
                for k in range(NJB):
                    j0 = k * JB
                    ni = K - j0
                    diff = work.tile([P, K, JB, D], _BF16, tag="diff")
                    in0 = (
                        ms[:, j0:K, :]
                        .unsqueeze(2)
                        .broadcast_to([P, ni, JB, D])
                    )
                    in1 = (
                        ms[:, j0 : j0 + JB, :]
                        .unsqueeze(1)
                        .broadcast_to([P, ni, JB, D])
                    )
                    dv = diff[:, :ni]
                    nc.vector.tensor_sub(dv, in0, in1)
                    nc.scalar.activation(
                        out=dv, in_=dv, func=mybir.ActivationFunctionType.Abs
                    )
                    l1 = small.tile([P, K, JB, 8], _BF16, tag="l1", bufs=2)
                    nc.vector.tensor_add(
                        l1[:, :ni], dv[:, :, :, 0:8], dv[:, :, :, 8:16]
                    )
                    l2 = small.tile([P, K, JB, 4], _BF16, tag="l2")
                    nc.vector.tensor_add(
                        l2[:, :ni], l1[:, :ni, :, 0:4], l1[:, :ni, :, 4:8]
                    )
                    l3 = small.tile([P, K, JB, 2], _BF16, tag="l3")
                    nc.vector.tensor_add(
                        l3[:, :ni], l2[:, :ni, :, 0:2], l2[:, :ni, :, 2:4]
                    )
                    norm = small.tile([P, K, JB], _F32, tag="norm")
                    nc.vector.tensor_add(
                        norm[:, :ni], l3[:, :ni, :, 0], l3[:, :ni, :, 1]
                    )
                    # exp per j-column; ScalarE accumulates the column sum
                    # sum_{i>=j0} E[i,j] directly into esum2[:, j]. Pairs in
                    # the diagonal block are attributed to O[j] only — the
                    # block is computed in both orders, so by symmetry O gets
                    # every unordered pair exactly once.
                    ee = small.tile([P, K, JB], _F32, tag="ee")
                    for jb in range(JB):
                        nc.scalar.activation(
                            out=ee[:, :ni, jb],
                            in_=norm[:, :ni, jb],
                            func=mybir.ActivationFunctionType.Exp,
                            scale=-1.0,
                            accum_out=esum2[:, j0 + jb : j0 + jb + 1],
                        )
                    # O[i] += sum_{j in block} E[i, j]  for cross rows only
                    if ni > JB:
                        nc.vector.tensor_reduce(
                            out=esum[:, j0 + JB : K, k : k + 1],
                            in_=ee[:, JB:ni, :],
                            axis=mybir.AxisListType.X,
                            op=mybir.AluOpType.add,
                        )

                o_tile = work.tile([P, K, 1], _F32, tag="o")
                nc.vector.tensor_reduce(
                    out=o_tile,
                    in_=esum,
                    axis=mybir.AxisListType.X,
                    op=mybir.AluOpType.add,
                )
                o2 = work.tile([P, K], _F32, tag="o2")
                nc.vector.tensor_add(o2, o_tile[:, :, 0], esum2)
                nc.sync.dma_start(out=out[t * P : (t + 1) * P, :], in_=o2)
    nc.compile()
    return nc


_cached = {}


def _get_nc():
    if "nc" not in _cached:
        _cached["nc"] = _build_nc()
    return _cached["nc"]


def kernel(x: np.ndarray, W: np.ndarray) -> np.ndarray:
    nc = _get_nc()
    xt = np.ascontiguousarray(x.T.astype(bfloat16))  # [F, B]
    wb = np.ascontiguousarray(W.astype(bfloat16))    # [F, ND]
    in_maps = [
        {
            "xt": np.ascontiguousarray(xt[:, c * BL : (c + 1) * BL]),
            "w": wb,
        }
        for c in range(NCORES)
    ]
    res = run_bass_kernel_spmd(nc, in_maps, core_ids=list(range(NCORES)))
    return np.concatenate(
        [res.results[c]["out"] for c in range(NCORES)], axis=0
    ).astype(np.float32)


# revision 18
# speedup vs baseline: 1.0208x; 1.0208x over previous
"""MinibatchDiscrimination kernel for Trainium2 (8 NeuronCores, SPMD).

Math: Ms = (x @ W).reshape(B, 128, 16)
      norm[b,i,j] = sum_d |Ms[b,i,d] - Ms[b,j,d]|
      out[b,i]    = sum_j exp(-norm[b,i,j])

Sharding: data-parallel over batch B across 8 cores (256 samples each).
W replicated. Host pre-casts to bf16 and pre-transposes x so the
stationary matmul operand loads without an on-device transpose.

Pairwise stage (per 128-sample tile, partitions = batch):
  - symmetry: for j-block [j0, j0+JB) compute only i in [j0, 128).
    Mirrored contributions for the cross region (i >= j0+JB) are added
    to O[j] via a transposed reduce.
  - DVE does the broadcast subtract at bf16 2x; ScalarE applies |.|
    in place; the d-reduction is a 4-level pairwise add tree on DVE
    (bf16 at 2x, final level fp32); ScalarE applies exp(-norm); DVE
    reduces over j into per-block slices, then once at the end.
"""

import os
import sys

sys.path.insert(0, "/opt/trn_rl_repo")
os.environ.setdefault("MYCRO_LOCAL_CACHE", "1")

import numpy as np
from ml_dtypes import bfloat16

import concourse.bacc as bacc
import concourse.bass as bass
import concourse.tile as tile
from concourse import mybir
from concourse.bass_utils import run_bass_kernel_spmd

B, F, K, D = 2048, 2048, 128, 16
NCORES = 8
BL = B // NCORES          # 256 rows per core
P = 128                   # partitions
NBT = BL // P             # 2 batch tiles per core
FB = F // P               # 16 contraction blocks
ND = K * D                # 2048 output cols of the matmul
JB = 4                    # j-block size in the pairwise stage
NJB = K // JB             # 32 j-blocks

_BF16 = mybir.dt.bfloat16
_F32 = mybir.dt.float32


def _build_nc():
    nc = bacc.Bacc("TRN2", target_bir_lowering=False, debug=False)
    xt = nc.dram_tensor("xt", [F, BL], _BF16, kind="ExternalInput")
    w = nc.dram_tensor("w", [F, ND], _BF16, kind="ExternalInput")
    out = nc.dram_tensor("out", [BL, K], _F32, kind="ExternalOutput")

    with tile.TileContext(nc) as tc:
        with (
            tc.tile_pool(name="const", bufs=1) as const_pool,
            tc.tile_pool(name="work", bufs=2) as work,
            tc.tile_pool(name="acc", bufs=1) as acc,
            tc.tile_pool(name="small", bufs=3) as small,
            tc.tile_pool(name="psum", bufs=2, space="PSUM") as psum_pool,
        ):
            w_sb = const_pool.tile([P, FB, ND], _BF16)
            xt_sb = const_pool.tile([P, FB, BL], _BF16)
            w_r = w.rearrange("(fb p) n -> p fb n", p=P)
            xt_r = xt.rearrange("(fb p) b -> p fb b", p=P)
            for fb in range(FB):
                nc.sync.dma_start(out=w_sb[:, fb, :], in_=w_r[:, fb, :])
                nc.sync.dma_start(out=xt_sb[:, fb, :], in_=xt_r[:, fb, :])

            for t in range(NBT):
                # ---- Ms = x @ W for this 128-sample tile ----
                ms = work.tile([P, K, D], _BF16, tag="ms")
                ms_flat = ms.rearrange("p k d -> p (k d)")
                psums = [
                    psum_pool.tile([P, 512], _F32, tag=f"ps{n}", name=f"ps{n}")
                    for n in range(4)
                ]
                for fb in range(FB):
                    lhsT = xt_sb[:, fb, t * P : (t + 1) * P]
                    for n in range(4):
                        nc.tensor.matmul(
                            psums[n],
                            lhsT,
                            w_sb[:, fb, n * 512 : (n + 1) * 512],
                            start=(fb == 0),
                            stop=(fb == FB - 1),
                        )
                for n in range(4):
                    nc.scalar.copy(
                        out=ms_flat[:, n * 512 : (n + 1) * 512], in_=psums[n][:, :]
                    )

                # ---- pairwise L1 + exp + sum over j (symmetric half) ----
                esum = acc.tile([P, K, NJB], _F32, tag="esum")
                esum2 = acc.tile([P, K], _F32, tag="esum2")
                nc.gpsimd.memset(esum, 0.0)
                nc.gpsimd.memset(esum2, 0.0)
                for k in range(NJB):
                    j0 = k * JB
                    ni = K - j0
                    diff = work.tile([P, K, JB, D], _BF16, tag="diff", bufs=3)
                    in0 = (
                        ms[:, j0:K, :]
                        .unsqueeze(2)
                        .broadcast_to([P, ni, JB, D])
                    )
                    in1 = (
                        ms[:, j0 : j0 + JB, :]
                        .unsqueeze(1)
                        .broadcast_to([P, ni, JB, D])
                    )
                    dv = diff[:, :ni]
                    nc.vector.tensor_sub(dv, in0, in1)
                    nc.scalar.activation(
                        out=dv, in_=dv, func=mybir.ActivationFunctionType.Abs
                    )
                    l1 = small.tile([P, K, JB, 8], _BF16, tag="l1", bufs=2)
                    nc.vector.tensor_add(
                        l1[:, :ni], dv[:, :, :, 0:8], dv[:, :, :, 8:16]
                    )
                    l2 = small.tile([P, K, JB, 4], _BF16, tag="l2")
                    nc.vector.tensor_add(
                        l2[:, :ni], l1[:, :ni, :, 0:4], l1[:, :ni, :, 4:8]
                    )
                    l3 = small.tile([P, K, JB, 2], _BF16, tag="l3")
                    nc.vector.tensor_add(
                        l3[:, :ni], l2[:, :ni, :, 0:2], l2[:, :ni, :, 2:4]
                    )
                    norm = small.tile([P, K, JB], _F32, tag="norm")
                    nc.vector.tensor_add(
                        norm[:, :ni], l3[:, :ni, :, 0], l3[:, :ni, :, 1]
                    )
                    ee = small.tile([P, K, JB], _F32, tag="ee")
                    nc.scalar.activation(
                        out=ee[:, :ni],
                        in_=norm[:, :ni],
                        func=mybir.ActivationFunctionType.Exp,
                        scale=-1.0,
                    )
                    # O[i] += sum_{j in block} E[i, j]   for i in [j0, K)
                    nc.vector.tensor_reduce(
                        out=esum[:, j0:K, k : k + 1],
                        in_=ee[:, :ni, :],
                        axis=mybir.AxisListType.X,
                        op=mybir.AluOpType.add,
                    )
                    # O[j] += sum_{i > j0+JB} E[i, j]  (mirror of cross pairs)
                    if ni > JB:
                        nc.vector.tensor_reduce(
                            out=esum2[:, j0 : j0 + JB].unsqueeze(2),
                            in_=ee[:, JB:ni, :].transpose([0, 2, 1]),
                            axis=mybir.AxisListType.X,
                            op=mybir.AluOpType.add,
                        )

                o_tile = work.tile([P, K, 1], _F32, tag="o")
                nc.vector.tensor_reduce(
                    out=o_tile,
                    in_=esum,
                    axis=mybir.AxisListType.X,
                    op=mybir.AluOpType.add,
                )
                o2 = work.tile([P, K], _F32, tag="o2")
                nc.vector.tensor_add(o2, o_tile[:, :, 0], esum2)
                nc.sync.dma_start(out=out[t * P : (t + 1) * P, :], in_=o2)
    nc.compile()
    return nc


_cached = {}


def _get_nc():
    if "nc" not in _cached:
        _cached["nc"] = _build_nc()
    return _cached["nc"]


def kernel(x: np.ndarray, W: np.ndarray) -> np.ndarray:
    nc = _get_nc()
    xt = np.ascontiguousarray(x.T.astype(bfloat16))  # [F, B]
    wb = np.ascontiguousarray(W.astype(bfloat16))    # [F, ND]
    in_maps = [
        {
            "xt": np.ascontiguousarray(xt[:, c * BL : (c + 1) * BL]),
            "w": wb,
        }
        for c in range(NCORES)
    ]
    res = run_bass_kernel_spmd(nc, in_maps, core_ids=list(range(NCORES)))
    return np.concatenate(
        [res.results[c]["out"] for c in range(NCORES)], axis=0
    ).astype(np.float32)


# revision 21
# speedup vs baseline: 1.0490x; 1.0277x over previous
"""MinibatchDiscrimination kernel for Trainium2 (8 NeuronCores, SPMD).

Math: Ms = (x @ W).reshape(B, 128, 16)
      norm[b,i,j] = sum_d |Ms[b,i,d] - Ms[b,j,d]|
      out[b,i]    = sum_j exp(-norm[b,i,j])

Sharding: data-parallel over batch B across 8 cores (256 samples each).
W replicated. Host pre-casts to bf16 and pre-transposes x so the
stationary matmul operand loads without an on-device transpose.

Pairwise stage (per 128-sample tile, partitions = batch):
  - symmetry: for j-block [j0, j0+JB) compute only i in [j0, 128).
    Mirrored contributions for the cross region (i >= j0+JB) are added
    to O[j] via a transposed reduce.
  - DVE does the broadcast subtract at bf16 2x; ScalarE applies |.|
    in place; the d-reduction is a 4-level pairwise add tree on DVE
    (bf16 at 2x, final level fp32); ScalarE applies exp(-norm); DVE
    reduces over j into per-block slices, then once at the end.
"""

import os
import sys

sys.path.insert(0, "/opt/trn_rl_repo")
os.environ.setdefault("MYCRO_LOCAL_CACHE", "1")

import numpy as np
from ml_dtypes import bfloat16

import concourse.bacc as bacc
import concourse.bass as bass
import concourse.tile as tile
from concourse import mybir
from concourse.bass_utils import run_bass_kernel_spmd

B, F, K, D = 2048, 2048, 128, 16
NCORES = 8
BL = B // NCORES          # 256 rows per core
P = 128                   # partitions
NBT = BL // P             # 2 batch tiles per core
FB = F // P               # 16 contraction blocks
ND = K * D                # 2048 output cols of the matmul
JB = 4                    # j-block size in the pairwise stage
NJB = K // JB             # 32 j-blocks

_BF16 = mybir.dt.bfloat16
_F32 = mybir.dt.float32


def _build_nc():
    nc = bacc.Bacc("TRN2", target_bir_lowering=False, debug=False)
    xt = nc.dram_tensor("xt", [F, BL], _BF16, kind="ExternalInput")
    w = nc.dram_tensor("w", [F, ND], _BF16, kind="ExternalInput")
    out = nc.dram_tensor("out", [BL, K], _F32, kind="ExternalOutput")

    with tile.TileContext(nc) as tc:
        with (
            tc.tile_pool(name="const", bufs=1) as const_pool,
            tc.tile_pool(name="work", bufs=2) as work,
            tc.tile_pool(name="acc", bufs=1) as acc,
            tc.tile_pool(name="small", bufs=3) as small,
            tc.tile_pool(name="psum", bufs=2, space="PSUM") as psum_pool,
        ):
            w_sb = const_pool.tile([P, FB, ND], _BF16)
            xt_sb = const_pool.tile([P, FB, BL], _BF16)
            w_r = w.rearrange("(fb p) n -> p fb n", p=P)
            xt_r = xt.rearrange("(fb p) b -> p fb b", p=P)
            for fb in range(FB):
                nc.sync.dma_start(out=xt_sb[:, fb, :], in_=xt_r[:, fb, :])
            # W arrives in n-quarters, last quarter first: the pairwise
            # stage consumes j-blocks in descending order, so compute can
            # start as soon as the tail quarter of Ms exists.
            for n in (3, 2, 1, 0):
                nc.sync.dma_start(
                    out=w_sb[:, :, n * 512 : (n + 1) * 512],
                    in_=w_r[:, :, n * 512 : (n + 1) * 512],
                )

            for t in range(NBT):
                # ---- Ms = x @ W for this 128-sample tile ----
                ms = work.tile([P, K, D], _BF16, tag="ms")
                ms_flat = ms.rearrange("p k d -> p (k d)")
                psums = [
                    psum_pool.tile([P, 512], _F32, tag=f"ps{n}", name=f"ps{n}")
                    for n in range(4)
                ]
                for n in (3, 2, 1, 0):
                    for fb in range(FB):
                        nc.tensor.matmul(
                            psums[n],
                            xt_sb[:, fb, t * P : (t + 1) * P],
                            w_sb[:, fb, n * 512 : (n + 1) * 512],
                            start=(fb == 0),
                            stop=(fb == FB - 1),
                        )
                    nc.scalar.copy(
                        out=ms_flat[:, n * 512 : (n + 1) * 512], in_=psums[n][:, :]
                    )

                # ---- pairwise L1 + exp + sum over j (symmetric half) ----
                esum = acc.tile([P, K, NJB], _F32, tag="esum")
                esum2 = acc.tile([P, K], _F32, tag="esum2")
                nc.gpsimd.memset(esum, 0.0)
                nc.gpsimd.memset(esum2, 0.0)
                for k in range(NJB - 1, -1, -1):
                    j0 = k * JB
                    ni = K - j0
                    diff = work.tile([P, K, JB, D], _BF16, tag="diff", bufs=3)
                    in0 = (
                        ms[:, j0:K, :]
                        .unsqueeze(2)
                        .broadcast_to([P, ni, JB, D])
                    )
                    in1 = (
                        ms[:, j0 : j0 + JB, :]
                        .unsqueeze(1)
                        .broadcast_to([P, ni, JB, D])
                    )
                    dv = diff[:, :ni]
                    nc.vector.tensor_sub(dv, in0, in1)
                    nc.scalar.activation(
                        out=dv, in_=dv, func=mybir.ActivationFunctionType.Abs
                    )
                    l1 = small.tile([P, K, JB, 8], _BF16, tag="l1", bufs=2)
                    nc.vector.tensor_add(
                        l1[:, :ni], dv[:, :, :, 0:8], dv[:, :, :, 8:16]
                    )
                    l2 = small.tile([P, K, JB, 4], _BF16, tag="l2")
                    nc.vector.tensor_add(
                        l2[:, :ni], l1[:, :ni, :, 0:4], l1[:, :ni, :, 4:8]
                    )
                    l3 = small.tile([P, K, JB, 2], _BF16, tag="l3")
                    nc.vector.tensor_add(
                        l3[:, :ni], l2[:, :ni, :, 0:2], l2[:, :ni, :, 2:4]
                    )
                    norm = small.tile([P, K, JB], _F32, tag="norm")
                    nc.vector.tensor_add(
                        norm[:, :ni], l3[:, :ni, :, 0], l3[:, :ni, :, 1]
                    )
                    ee = small.tile([P, K, JB], _F32, tag="ee")
                    nc.scalar.activation(
                        out=ee[:, :ni],
                        in_=norm[:, :ni],
                        func=mybir.ActivationFunctionType.Exp,
                        scale=-1.0,
                    )
                    # O[i] += sum_{j in block} E[i, j]   for i in [j0, K)
                    nc.vector.tensor_reduce(
                        out=esum[:, j0:K, k : k + 1],
                        in_=ee[:, :ni, :],
                        axis=mybir.AxisListType.X,
                        op=mybir.AluOpType.add,
                    )
                    # O[j] += sum_{i > j0+JB} E[i, j]  (mirror of cross pairs)
                    if ni > JB:
                        nc.vector.tensor_reduce(
                            out=esum2[:, j0 : j0 + JB].unsqueeze(2),
                            in_=ee[:, JB:ni, :].transpose([0, 2, 1]),
                            axis=mybir.AxisListType.X,
                            op=mybir.AluOpType.add,
                        )

                o_tile = work.tile([P, K, 1], _F32, tag="o")
                nc.vector.tensor_reduce(
                    out=o_tile,
                    in_=esum,
                    axis=mybir.AxisListType.X,
                    op=mybir.AluOpType.add,
                )
                o2 = work.tile([P, K], _F32, tag="o2")
                nc.vector.tensor_add(o2, o_tile[:, :, 0], esum2)
                nc.sync.dma_start(out=out[t * P : (t + 1) * P, :], in_=o2)
    nc.compile()
    return nc


_cached = {}


def _get_nc():
    if "nc" not in _cached:
        _cached["nc"] = _build_nc()
    return _cached["nc"]


def kernel(x: np.ndarray, W: np.ndarray) -> np.ndarray:
    nc = _get_nc()
    xt = np.ascontiguousarray(x.T.astype(bfloat16))  # [F, B]
    wb = np.ascontiguousarray(W.astype(bfloat16))    # [F, ND]
    in_maps = [
        {
            "xt": np.ascontiguousarray(xt[:, c * BL : (c + 1) * BL]),
            "w": wb,
        }
        for c in range(NCORES)
    ]
    res = run_bass_kernel_spmd(nc, in_maps, core_ids=list(range(NCORES)))
    return np.concatenate(
        [res.results[c]["out"] for c in range(NCORES)], axis=0
    ).astype(np.float32)


# revision 23
# speedup vs baseline: 1.0571x; 1.0077x over previous
"""MinibatchDiscrimination kernel for Trainium2 (8 NeuronCores, SPMD).

Math: Ms = (x @ W).reshape(B, 128, 16)
      norm[b,i,j] = sum_d |Ms[b,i,d] - Ms[b,j,d]|
      out[b,i]    = sum_j exp(-norm[b,i,j])

Sharding: data-parallel over batch B across 8 cores (256 samples each).
W replicated. Host pre-casts to bf16 and pre-transposes x so the
stationary matmul operand loads without an on-device transpose.

Pairwise stage (per 128-sample tile, partitions = batch):
  - symmetry: for j-block [j0, j0+JB) compute only i in [j0, 128).
    Mirrored contributions for the cross region (i >= j0+JB) are added
    to O[j] via a transposed reduce.
  - DVE does the broadcast subtract at bf16 2x; ScalarE applies |.|
    in place; the d-reduction is a 4-level pairwise add tree on DVE
    (bf16 at 2x, final level fp32); ScalarE applies exp(-norm); DVE
    reduces over j into per-block slices, then once at the end.
"""

import os
import sys

sys.path.insert(0, "/opt/trn_rl_repo")
os.environ.setdefault("MYCRO_LOCAL_CACHE", "1")

import numpy as np
from ml_dtypes import bfloat16

import concourse.bacc as bacc
import concourse.bass as bass
import concourse.tile as tile
from concourse import mybir
from concourse.bass_utils import run_bass_kernel_spmd

B, F, K, D = 2048, 2048, 128, 16
NCORES = 8
BL = B // NCORES          # 256 rows per core
P = 128                   # partitions
NBT = BL // P             # 2 batch tiles per core
FB = F // P               # 16 contraction blocks
ND = K * D                # 2048 output cols of the matmul
JB = 4                    # j-block size in the pairwise stage
NJB = K // JB             # 32 j-blocks

_BF16 = mybir.dt.bfloat16
_F32 = mybir.dt.float32


def _build_nc():
    nc = bacc.Bacc("TRN2", target_bir_lowering=False, debug=False)
    xt = nc.dram_tensor("xt", [F, BL], _BF16, kind="ExternalInput")
    w = nc.dram_tensor("w", [F, ND], _BF16, kind="ExternalInput")
    out = nc.dram_tensor("out", [BL, K], _F32, kind="ExternalOutput")

    with tile.TileContext(nc) as tc:
        with (
            tc.tile_pool(name="const", bufs=1) as const_pool,
            tc.tile_pool(name="work", bufs=2) as work,
            tc.tile_pool(name="acc", bufs=1) as acc,
            tc.tile_pool(name="small", bufs=3) as small,
            tc.tile_pool(name="psum", bufs=2, space="PSUM") as psum_pool,
        ):
            w_sb = const_pool.tile([P, FB, ND], _BF16)
            xt_sb = const_pool.tile([P, FB, BL], _BF16)
            w_r = w.rearrange("(fb p) n -> p fb n", p=P)
            xt_r = xt.rearrange("(fb p) b -> p fb b", p=P)
            for fb in range(FB):
                nc.sync.dma_start(out=xt_sb[:, fb, :], in_=xt_r[:, fb, :])
            # W arrives in n-quarters, last quarter first: the pairwise
            # stage consumes j-blocks in descending order, so compute can
            # start as soon as the tail quarter of Ms exists.
            for c0, c1 in [(1792, 2048), (1536, 1792), (1024, 1536),
                           (512, 1024), (0, 512)]:
                nc.sync.dma_start(
                    out=w_sb[:, :, c0:c1], in_=w_r[:, :, c0:c1]
                )

            for t in range(NBT):
                # ---- Ms = x @ W for this 128-sample tile ----
                ms = work.tile([P, K, D], _BF16, tag="ms")
                ms_flat = ms.rearrange("p k d -> p (k d)")
                psums = [
                    psum_pool.tile([P, 512], _F32, tag=f"ps{n}", name=f"ps{n}")
                    for n in range(4)
                ]
                # tail quarter computed in two 256-col chunks so the first
                # (descending) j-blocks can start as early as possible
                chunks = [(1792, 2048), (1536, 1792), (1024, 1536),
                          (512, 1024), (0, 512)]
                for c0, c1 in chunks:
                    n = c0 // 512
                    for fb in range(FB):
                        nc.tensor.matmul(
                            psums[n][:, c0 - n * 512 : c1 - n * 512],
                            xt_sb[:, fb, t * P : (t + 1) * P],
                            w_sb[:, fb, c0:c1],
                            start=(fb == 0),
                            stop=(fb == FB - 1),
                        )
                    nc.scalar.copy(
                        out=ms_flat[:, c0:c1],
                        in_=psums[n][:, c0 - n * 512 : c1 - n * 512],
                    )

                # ---- pairwise L1 + exp + sum over j (symmetric half) ----
                esum = acc.tile([P, K, NJB], _F32, tag="esum")
                esum2 = acc.tile([P, K], _F32, tag="esum2")
                nc.gpsimd.memset(esum, 0.0)
                nc.gpsimd.memset(esum2, 0.0)
                for k in range(NJB - 1, -1, -1):
                    j0 = k * JB
                    ni = K - j0
                    diff = work.tile([P, K, JB, D], _BF16, tag="diff", bufs=3)
                    in0 = (
                        ms[:, j0:K, :]
                        .unsqueeze(2)
                        .broadcast_to([P, ni, JB, D])
                    )
                    in1 = (
                        ms[:, j0 : j0 + JB, :]
                        .unsqueeze(1)
                        .broadcast_to([P, ni, JB, D])
                    )
                    dv = diff[:, :ni]
                    nc.vector.tensor_sub(dv, in0, in1)
                    nc.scalar.activation(
                        out=dv, in_=dv, func=mybir.ActivationFunctionType.Abs
                    )
                    l1 = small.tile([P, K, JB, 8], _BF16, tag="l1", bufs=2)
                    nc.vector.tensor_add(
                        l1[:, :ni], dv[:, :, :, 0:8], dv[:, :, :, 8:16]
                    )
                    l2 = small.tile([P, K, JB, 4], _BF16, tag="l2")
                    nc.vector.tensor_add(
                        l2[:, :ni], l1[:, :ni, :, 0:4], l1[:, :ni, :, 4:8]
                    )
                    l3 = small.tile([P, K, JB, 2], _BF16, tag="l3")
                    nc.vector.tensor_add(
                        l3[:, :ni], l2[:, :ni, :, 0:2], l2[:, :ni, :, 2:4]
                    )
                    norm = small.tile([P, K, JB], _F32, tag="norm")
                    nc.vector.tensor_add(
                        norm[:, :ni], l3[:, :ni, :, 0], l3[:, :ni, :, 1]
                    )
                    ee = small.tile([P, K, JB], _F32, tag="ee")
                    nc.scalar.activation(
                        out=ee[:, :ni],
                        in_=norm[:, :ni],
                        func=mybir.ActivationFunctionType.Exp,
                        scale=-1.0,
                    )
                    # O[i] += sum_{j in block} E[i, j]   for i in [j0, K)
                    nc.vector.tensor_reduce(
                        out=esum[:, j0:K, k : k + 1],
                        in_=ee[:, :ni, :],
                        axis=mybir.AxisListType.X,
                        op=mybir.AluOpType.add,
                    )
                    # O[j] += sum_{i > j0+JB} E[i, j]  (mirror of cross pairs)
                    if ni > JB:
                        nc.vector.tensor_reduce(
                            out=esum2[:, j0 : j0 + JB].unsqueeze(2),
                            in_=ee[:, JB:ni, :].transpose([0, 2, 1]),
                            axis=mybir.AxisListType.X,
                            op=mybir.AluOpType.add,
                        )

                o_tile = work.tile([P, K, 1], _F32, tag="o")
                nc.vector.tensor_reduce(
                    out=o_tile,
                    in_=esum,
                    axis=mybir.AxisListType.X,
                    op=mybir.AluOpType.add,
                )
                o2 = work.tile([P, K], _F32, tag="o2")
                nc.vector.tensor_add(o2, o_tile[:, :, 0], esum2)
                nc.sync.dma_start(out=out[t * P : (t + 1) * P, :], in_=o2)
    nc.compile()
    return nc


_cached = {}


def _get_nc():
    if "nc" not in _cached:
        _cached["nc"] = _build_nc()
    return _cached["nc"]


def kernel(x: np.ndarray, W: np.ndarray) -> np.ndarray:
    nc = _get_nc()
    xt = np.ascontiguousarray(x.T.astype(bfloat16))  # [F, B]
    wb = np.ascontiguousarray(W.astype(bfloat16))    # [F, ND]
    in_maps = [
        {
            "xt": np.ascontiguousarray(xt[:, c * BL : (c + 1) * BL]),
            "w": wb,
        }
        for c in range(NCORES)
    ]
    res = run_bass_kernel_spmd(nc, in_maps, core_ids=list(range(NCORES)))
    return np.concatenate(
        [res.results[c]["out"] for c in range(NCORES)], axis=0
    ).astype(np.float32)


# revision 24
# speedup vs baseline: 1.0621x; 1.0047x over previous
"""MinibatchDiscrimination kernel for Trainium2 (8 NeuronCores, SPMD).

Math: Ms = (x @ W).reshape(B, 128, 16)
      norm[b,i,j] = sum_d |Ms[b,i,d] - Ms[b,j,d]|
      out[b,i]    = sum_j exp(-norm[b,i,j])

Sharding: data-parallel over batch B across 8 cores (256 samples each).
W replicated. Host pre-casts to bf16 and pre-transposes x so the
stationary matmul operand loads without an on-device transpose.

Pairwise stage (per 128-sample tile, partitions = batch):
  - symmetry: for j-block [j0, j0+JB) compute only i in [j0, 128).
    Mirrored contributions for the cross region (i >= j0+JB) are added
    to O[j] via a transposed reduce.
  - DVE does the broadcast subtract at bf16 2x; ScalarE applies |.|
    in place; the d-reduction is a 4-level pairwise add tree on DVE
    (bf16 at 2x, final level fp32); ScalarE applies exp(-norm); DVE
    reduces over j into per-block slices, then once at the end.
"""

import os
import sys

sys.path.insert(0, "/opt/trn_rl_repo")
os.environ.setdefault("MYCRO_LOCAL_CACHE", "1")

import numpy as np
from ml_dtypes import bfloat16

import concourse.bacc as bacc
import concourse.bass as bass
import concourse.tile as tile
from concourse import mybir
from concourse.bass_utils import run_bass_kernel_spmd

B, F, K, D = 2048, 2048, 128, 16
NCORES = 8
BL = B // NCORES          # 256 rows per core
P = 128                   # partitions
NBT = BL // P             # 2 batch tiles per core
FB = F // P               # 16 contraction blocks
ND = K * D                # 2048 output cols of the matmul
JB = 4                    # j-block size in the pairwise stage
NJB = K // JB             # 32 j-blocks

_BF16 = mybir.dt.bfloat16
_F32 = mybir.dt.float32


def _build_nc():
    nc = bacc.Bacc("TRN2", target_bir_lowering=False, debug=False)
    xt = nc.dram_tensor("xt", [F, BL], _BF16, kind="ExternalInput")
    w = nc.dram_tensor("w", [F, ND], _BF16, kind="ExternalInput")
    out = nc.dram_tensor("out", [BL, K], _F32, kind="ExternalOutput")

    with tile.TileContext(nc) as tc:
        with (
            tc.tile_pool(name="const", bufs=1) as const_pool,
            tc.tile_pool(name="work", bufs=2) as work,
            tc.tile_pool(name="acc", bufs=1) as acc,
            tc.tile_pool(name="small", bufs=3) as small,
            tc.tile_pool(name="psum", bufs=2, space="PSUM") as psum_pool,
        ):
            w_sb = const_pool.tile([P, FB, ND], _BF16)
            xt_sb = const_pool.tile([P, FB, BL], _BF16)
            w_r = w.rearrange("(fb p) n -> p fb n", p=P)
            xt_r = xt.rearrange("(fb p) b -> p fb b", p=P)
            for fb in range(FB):
                nc.sync.dma_start(out=xt_sb[:, fb, :], in_=xt_r[:, fb, :])
            # W arrives in n-quarters, last quarter first: the pairwise
            # stage consumes j-blocks in descending order, so compute can
            # start as soon as the tail quarter of Ms exists.
            for c0, c1 in [(1792, 2048), (1536, 1792), (1024, 1536),
                           (512, 1024), (0, 512)]:
                nc.sync.dma_start(
                    out=w_sb[:, :, c0:c1], in_=w_r[:, :, c0:c1]
                )

            for t in range(NBT):
                # ---- Ms = x @ W for this 128-sample tile ----
                ms = work.tile([P, K, D], _BF16, tag="ms")
                ms_flat = ms.rearrange("p k d -> p (k d)")
                psums = [
                    psum_pool.tile([P, 512], _F32, tag=f"ps{n}", name=f"ps{n}")
                    for n in range(4)
                ]
                # tail quarter computed in two 256-col chunks so the first
                # (descending) j-blocks can start as early as possible
                chunks = [(1792, 2048), (1536, 1792), (1024, 1536),
                          (512, 1024), (0, 512)]
                for c0, c1 in chunks:
                    n = c0 // 512
                    for fb in range(FB):
                        nc.tensor.matmul(
                            psums[n][:, c0 - n * 512 : c1 - n * 512],
                            xt_sb[:, fb, t * P : (t + 1) * P],
                            w_sb[:, fb, c0:c1],
                            start=(fb == 0),
                            stop=(fb == FB - 1),
                        )
                    nc.scalar.copy(
                        out=ms_flat[:, c0:c1],
                        in_=psums[n][:, c0 - n * 512 : c1 - n * 512],
                    )

                # ---- pairwise L1 + exp + sum over j (symmetric half) ----
                esum = acc.tile([P, K, NJB], _F32, tag="esum")
                esum2 = acc.tile([P, K], _F32, tag="esum2")
                nc.gpsimd.memset(esum, 0.0)
                nc.gpsimd.memset(esum2, 0.0)
                # j-block specs (j0, width), descending j0. Tail blocks
                # (j0 >= 64) are merged to width 8 — their element counts
                # still fit the flat tiles, and the attribution scheme is
                # width-agnostic (wider diagonal block, both orders
                # computed).
                specs = [(j0, 8) for j0 in range(120, 56, -8)] + [
                    (j0, 4) for j0 in range(60, -1, -4)
                ]
                for si, (j0, w) in enumerate(specs):
                    ni = K - j0
                    diff = work.tile([P, K * JB * D], _BF16, tag="diff",
                                     bufs=3, name="diff")
                    in0 = (
                        ms[:, j0:K, :]
                        .unsqueeze(2)
                        .broadcast_to([P, ni, w, D])
                    )
                    in1 = (
                        ms[:, j0 : j0 + w, :]
                        .unsqueeze(1)
                        .broadcast_to([P, ni, w, D])
                    )
                    dv = diff[:, : ni * w * D].rearrange(
                        "p (i j d) -> p i j d", j=w, d=D
                    )
                    nc.vector.tensor_sub(dv, in0, in1)
                    nc.scalar.activation(
                        out=dv, in_=dv, func=mybir.ActivationFunctionType.Abs
                    )
                    l1f = small.tile([P, K * JB * 8], _BF16, tag="l1",
                                     bufs=2, name="l1")
                    l1 = l1f[:, : ni * w * 8].rearrange(
                        "p (i j d) -> p i j d", j=w, d=8
                    )
                    nc.vector.tensor_add(l1, dv[:, :, :, 0:8], dv[:, :, :, 8:16])
                    l2f = small.tile([P, K * JB * 4], _BF16, tag="l2",
                                     name="l2")
                    l2 = l2f[:, : ni * w * 4].rearrange(
                        "p (i j d) -> p i j d", j=w, d=4
                    )
                    nc.vector.tensor_add(l2, l1[:, :, :, 0:4], l1[:, :, :, 4:8])
                    l3f = small.tile([P, K * JB * 2], _BF16, tag="l3",
                                     name="l3")
                    l3 = l3f[:, : ni * w * 2].rearrange(
                        "p (i j d) -> p i j d", j=w, d=2
                    )
                    nc.vector.tensor_add(l3, l2[:, :, :, 0:2], l2[:, :, :, 2:4])
                    normf = small.tile([P, K * JB], _F32, tag="norm",
                                       name="norm")
                    norm = normf[:, : ni * w].rearrange(
                        "p (i j) -> p i j", j=w
                    )
                    nc.vector.tensor_add(norm, l3[:, :, :, 0], l3[:, :, :, 1])
                    eef = small.tile([P, K * JB], _F32, tag="ee", name="ee")
                    ee = eef[:, : ni * w].rearrange("p (i j) -> p i j", j=w)
                    nc.scalar.activation(
                        out=ee,
                        in_=norm,
                        func=mybir.ActivationFunctionType.Exp,
                        scale=-1.0,
                    )
                    # O[i] += sum_{j in block} E[i, j]   for i in [j0, K)
                    nc.vector.tensor_reduce(
                        out=esum[:, j0:K, si : si + 1],
                        in_=ee,
                        axis=mybir.AxisListType.X,
                        op=mybir.AluOpType.add,
                    )
                    # O[j] += sum_{i > j0+w} E[i, j]  (mirror of cross pairs)
                    if ni > w:
                        nc.vector.tensor_reduce(
                            out=esum2[:, j0 : j0 + w].unsqueeze(2),
                            in_=ee[:, w:ni, :].transpose([0, 2, 1]),
                            axis=mybir.AxisListType.X,
                            op=mybir.AluOpType.add,
                        )

                o_tile = work.tile([P, K, 1], _F32, tag="o")
                nc.vector.tensor_reduce(
                    out=o_tile,
                    in_=esum,
                    axis=mybir.AxisListType.X,
                    op=mybir.AluOpType.add,
                )
                o2 = work.tile([P, K], _F32, tag="o2")
                nc.vector.tensor_add(o2, o_tile[:, :, 0], esum2)
                nc.sync.dma_start(out=out[t * P : (t + 1) * P, :], in_=o2)
    nc.compile()
    return nc


_cached = {}


def _get_nc():
    if "nc" not in _cached:
        _cached["nc"] = _build_nc()
    return _cached["nc"]


def kernel(x: np.ndarray, W: np.ndarray) -> np.ndarray:
    nc = _get_nc()
    xt = np.ascontiguousarray(x.T.astype(bfloat16))  # [F, B]
    wb = np.ascontiguousarray(W.astype(bfloat16))    # [F, ND]
    in_maps = [
        {
            "xt": np.ascontiguousarray(xt[:, c * BL : (c + 1) * BL]),
            "w": wb,
        }
        for c in range(NCORES)
    ]
    res = run_bass_kernel_spmd(nc, in_maps, core_ids=list(range(NCORES)))
    return np.concatenate(
        [res.results[c]["out"] for c in range(NCORES)], axis=0
    ).astype(np.float32)
